# revision 1
# baseline (speedup 1.0000x reference)
# Trainium2 Bass kernel for nn_Block_SA (dense_cnn self-attention block).
#
# Per-sample computation (C=64 channels, 64x64 spatial, N=4096 positions):
#   v   = relu(bn1(conv1x1(x)))                      # V for attention
#   s   = (x^T x) / sqrt(C)                          # [N, N] scores, Q=K=x
#   p   = softmax(s, axis=-1)
#   a   = V p^T  (a[d,n] = sum_m p[n,m] V[d,m])
#   z   = relu(bn2(depthwise3x3(a)))
#   out = bn3(conv1x1(z)) + x
#
# Distribution: batch B=8, one sample per NeuronCore (data parallel, no
# collectives). BN params are folded into conv weights on the host.
#
# On-chip algorithm (per core):
#   - Scores are computed TRANSPOSED: sT[m, n] tiles via matmul(lhsT=x[:,mtile],
#     rhs=x[:,nchunk]) so softmax's sum over m becomes a matmul reduction.
#     The K=64 contraction uses only half the PE array, so score matmuls are
#     row-packed two-at-a-time with tile_position (x duplicated on partitions
#     64-127) for ~2x PE throughput.
#   - exp() without max subtraction (scores/8 are small; fp32 is safe).
#   - The denominator sum_m exp(sT[m,n]) is obtained for free by augmenting
#     V^T with a ones column (row 64 of the AV accumulator).
#   - AV accumulates over all 32 m-tiles into one PSUM bank; 1/den is
#     broadcast across partitions with a K=1 PE matmul against a ones row.
#   - Depthwise 3x3 runs on the PE as 9 accumulating diagonal-weight matmuls
#     over shifted views of the attention output (PSUM accumulates taps);
#     the vector engine only applies bias+relu.
#   - conv3 + bias via augmented ones row; residual add; DMA out.
#
# Matmuls use float32r (full-rate fp32 mode on TRN2's PE; fp32 proper is
# 4 cycles/row). f32r operands must be produced by rounding-capable engines
# (DVE/ACT writes), not plain DMA.

import numpy as np

_EPS = 1e-5
_C = 64
_CP1 = 65
_N = 4096
_CH = 512          # free-dim chunk (one PSUM bank of fp32)
_NCH = _N // _CH   # 8 chunks
_MT = 128          # m-tile (partition dim of transposed score tiles)
_NMT = _N // _MT   # 32 m-tiles
_W = 64            # image width
_GROUPS = [3] * 10 + [2]   # m-tiles per exp batch (3 PSUM banks per batch)
_NCONST = 138 + 9 * 64 + 9  # w1aug | w3aug | w2p | b2p | diags | -w2p
_ROW_PACK = True   # row-pack K=64 score matmuls via tile_position

_STATE = {}


def _build_program(reps=1):
    import concourse.bacc as bacc
    import concourse.tile as tile
    from concourse import mybir

    F32 = mybir.dt.float32
    F32R = mybir.dt.float32r
    U32 = mybir.dt.uint32
    AF = mybir.ActivationFunctionType
    ALU = mybir.AluOpType
    ONE_BITS = 0x3F800000

    nc = bacc.Bacc(None)

    xd = nc.dram_tensor("x", [_C, _N], F32, kind="ExternalInput")
    # packed weights -> one DMA: cols 0:64 w1aug, 64:128 w3aug,
    # 128:137 w2p (rows 0:64), 137 b2p, 138:714 diag(w2p[:,k]) k=0..8
    cd = nc.dram_tensor("consts", [_CP1, _NCONST], F32, kind="ExternalInput")
    outd = nc.dram_tensor("out", [_C, _N], F32, kind="ExternalOutput")

    with tile.TileContext(nc) as tc:
        with (
            tc.tile_pool(name="persist", bufs=1) as pp,
            tc.tile_pool(name="small", bufs=2) as sp,
            tc.tile_pool(name="pt_pool", bufs=3) as ptp,
            tc.tile_pool(name="ps_pool", bufs=2, space="PSUM") as psp,
            tc.tile_pool(name="po_pool", bufs=1, space="PSUM") as pop,
            tc.tile_pool(name="aux_pool", bufs=1, space="PSUM") as auxp,
        ):
            def emit_all():
                # ---- input staging: x first (critical path). x is loaded twice
                # (partitions 0:64 and 64:128) so score matmuls can row-pack.
                xs2 = pp.tile([_MT, _N], F32, name="xs2", tag="xs2")
                xr2 = pp.tile([_MT, _N], F32R, name="xr2", tag="xr2")
                for s in range(4):
                    sl = slice(1024 * s, 1024 * (s + 1))
                    nc.sync.dma_start(xs2[0:_C, sl], xd[:, sl])
                    nc.sync.dma_start(xs2[_C:_MT, sl], xd[:, sl])
                    nc.vector.tensor_copy(xr2[:, sl], xs2[:, sl])

                cs = pp.tile([_CP1, _NCONST], F32, name="cs", tag="cs")
                nc.scalar.dma_start(cs[:], cd[:])
                w1s = cs[:, 0:64]
                w2s = cs[0:_C, 128:137]
                b2s = cs[0:_C, 137:138]
                w2n = cs[0:_C, 714:723]

                # x + ones row (fp32): VT matmul lhsT and the residual add
                xo = pp.tile([_CP1, _N], F32, name="xo", tag="xo")
                nc.sync.dma_start(xo[0:_C, :], xd[:])
                nc.gpsimd.memset(xo[_C:_CP1, :], 1.0)

                # f32r-rounded weight copies
                w3r = pp.tile([_CP1, _C], F32R, name="w3r", tag="w3r")
                nc.vector.tensor_copy(w3r[:], cs[:, 64:128])
                dgr = pp.tile([_C, 9 * _C], F32R, name="dgr", tag="dgr")
                nc.vector.tensor_copy(dgr[:], cs[0:_C, 138:138 + 9 * _C])
                ones_r = pp.tile([1, _C], F32R, name="ones_r", tag="ones_r")
                nc.vector.memset(ones_r[:].bitcast(U32), ONE_BITS)

                # V^T blocks: per m-tile a [128, 65] block (col 64 = ones)
                vt = pp.tile([_MT, _NMT * _CP1], F32R, name="vt", tag="vt")
                vt3 = vt.rearrange("p (t c) -> p t c", c=_CP1)
                nc.gpsimd.memset(vt3[:, :, _C:_CP1].bitcast(U32), ONE_BITS)

                # normalized attention output (f32r: feeds the PE depthwise).
                # One zeroed pad row of 64 on each side so flat row-spanning
                # shifted reads stay in bounds.
                yrp = pp.tile([_C, _N + 2 * _W], F32R, name="yrp", tag="yrp")
                nc.gpsimd.memset(yrp[:, 0:_W].bitcast(U32), 0)
                nc.gpsimd.memset(yrp[:, _W + _N : _N + 2 * _W].bitcast(U32), 0)
                yr = yrp[:, _W : _W + _N]
                # post-depthwise activations (+ones row) feeding conv3
                zr = pp.tile([_CP1, _N], F32R, name="zr", tag="zr")
                nc.gpsimd.memset(zr[_C:_CP1, :].bitcast(U32), ONE_BITS)
                zrv = zr[0:_C, :].rearrange("c (h w) -> c h w", w=_W)

                # ---- V^T groups: emitted lazily (interleaved into chunk 0's
                # group loop) so the cold PE isn't blocked on them at startup.
                # relu on DVE (not ACT) so the scalar engine runs Exp only.
                _vt_emitted = [0]

                def emit_vt_groups(need_mtiles):
                    while _vt_emitted[0] * 4 < need_mtiles:
                        g = _vt_emitted[0]
                        vps = auxp.tile([_MT, 4 * _C], F32, name="vps", tag="aux")
                        for j in range(4):
                            m = 4 * g + j
                            nc.tensor.matmul(
                                vps[:, _C * j : _C * (j + 1)],
                                lhsT=xo[:, _MT * m : _MT * (m + 1)],
                                rhs=w1s,
                                start=True,
                                stop=True,
                            )
                        nc.vector.tensor_relu(
                            vt3[:, 4 * g : 4 * (g + 1), 0:_C],
                            vps[:].rearrange("p (t c) -> p t c", c=_C),
                        )
                        _vt_emitted[0] += 1

                # ---- depthwise 3x3 on the PE: 9 accumulating diagonal matmuls
                # over flat shifted views of yr (full-row spans; the dx=+-1 taps
                # wrap across row edges and get small DVE fix-ups that subtract
                # the wrong contributions), then bias+relu on DVE.
                yrp3 = yrp.rearrange("c (h w) -> c h w", w=_W)  # row i = y row i-1

                def emit_dw(h0, h1):
                    nh = h1 - h0
                    dwp = auxp.tile([_C, nh * _W], F32, name="dwp", tag="aux")
                    dwp3 = dwp.rearrange("c (h w) -> c h w", w=_W)
                    mms = []      # (k, out_lo, out_hi, src_lo, src_hi)
                    fixups = []
                    lastrow = []
                    for k in [4, 0, 1, 2, 3, 5, 6, 7, 8]:
                        dy, dx = k // 3 - 1, k % 3 - 1
                        hh0, hh1 = max(h0, -dy), min(h1, _W - dy)
                        if hh1 <= hh0:
                            continue
                        # tap k=8's flat span would read one element of the NEXT
                        # chunk's y (row hh1+1, w=0): do its last row exactly
                        flat_hh1 = hh1 - 1 if k == 8 else hh1
                        nhh = flat_hh1 - hh0
                        if nhh > 0:
                            src = _W + (hh0 + dy) * _W + dx
                            mms.append((k, (hh0 - h0) * _W, (flat_hh1 - h0) * _W,
                                        src, src + nhh * _W))
                            if dx != 0:
                                fixups.append((k, dy, dx, hh0, nhh))
                        if k == 8:
                            # f32r matmuls need even free counts; this 63-wide
                            # row goes on the DVE instead (after the PE group)
                            lastrow.append(hh1 - 1)
                    for i, (k, o0, o1, s0, s1) in enumerate(mms):
                        nc.tensor.matmul(
                            dwp[:, o0:o1],
                            lhsT=dgr[:, _C * k : _C * (k + 1)],
                            rhs=yrp[:, s0:s1],
                            start=(i == 0),
                            stop=(i == len(mms) - 1),
                            skip_group_check=True,
                        )
                    # tap k=8's last row: out[h, 0:63] += w2[8]*y[h+1, 1:64)
                    for h in lastrow:
                        dst8 = dwp3[:, h - h0 : h - h0 + 1, 0 : _W - 1]
                        nc.vector.scalar_tensor_tensor(
                            dst8,
                            yrp3[:, h + 2 : h + 3, 1:_W],
                            w2s[:, 8:9],
                            dst8,
                            op0=ALU.mult,
                            op1=ALU.add,
                        )
                    # subtract the wrap-around contribution of the dx=+-1 taps:
                    # dx=-1 polluted w=0 (read prev flat row's w=63), dx=+1
                    # polluted w=63 (read next flat row's w=0)
                    for k, dy, dx, hh0, nhh in fixups:
                        if dx == -1:
                            dst = dwp3[:, hh0 - h0 : hh0 - h0 + nhh, 0:1]
                            bad = yrp3[:, hh0 + dy : hh0 + dy + nhh, _W - 1 : _W]
                        else:
                            dst = dwp3[:, hh0 - h0 : hh0 - h0 + nhh, _W - 1 : _W]
                            bad = yrp3[:, hh0 + dy + 2 : hh0 + dy + 2 + nhh, 0:1]
                        nc.vector.scalar_tensor_tensor(
                            dst, bad, w2n[:, k : k + 1], dst, op0=ALU.mult, op1=ALU.add
                        )
                    nc.vector.tensor_scalar(
                        zrv[:, h0:h1, :], dwp3[:], b2s, 0.0, op0=ALU.add, op1=ALU.max
                    )

                def emit_conv3(c):
                    # conv3 (+bias via ones row) + residual + store
                    pc = auxp.tile([_C, _CH], F32, name="pc", tag="aux")
                    nc.tensor.matmul(
                        pc[:],
                        lhsT=w3r[:],
                        rhs=zr[:, _CH * c : _CH * (c + 1)],
                        start=True,
                        stop=True,
                    )
                    outt = sp.tile([_C, _CH], F32, name="outt", tag="outt", bufs=2)
                    nc.vector.tensor_tensor(
                        outt[:], pc[:], xo[0:_C, _CH * c : _CH * (c + 1)], op=ALU.add
                    )
                    nc.sync.dma_start(outd[:, _CH * c : _CH * (c + 1)], outt[:])

                # ---- main fused-attention loop over n-chunks ----
                for ci in range(_NCH):
                    po = pop.tile([_MT, _CH], F32, name="po", tag="po")
                    m = 0
                    # chunk 0 leads with small groups so the scalar engine's exp
                    # stream starts as soon as the first score tile exists
                    groups = ([1, 2] + [3] * 9 + [2]) if ci == 0 else _GROUPS
                    for msz in groups:
                        ps = psp.tile([_MT, _CH * msz], F32, name="ps", tag="ps")
                        for j in range(msz):
                            half = (m + j) % 2 if _ROW_PACK else 0
                            rows = slice(_C * half, _C * (half + 1))
                            nc.tensor.matmul(
                                ps[:, _CH * j : _CH * (j + 1)],
                                lhsT=xr2[rows, _MT * (m + j) : _MT * (m + j + 1)],
                                rhs=xr2[rows, _CH * ci : _CH * (ci + 1)],
                                start=True,
                                stop=True,
                                tile_position=(_C * half, 0) if _ROW_PACK else None,
                            )
                        pt = ptp.tile([_MT, _CH * msz], F32R, name="pt", tag="pt")
                        nc.scalar.activation(pt[:], ps[:], AF.Exp, scale=0.125)
                        emit_vt_groups(m + msz)
                        for j in range(msz):
                            nc.tensor.matmul(
                                po[0:_CP1, :],
                                lhsT=vt[:, _CP1 * (m + j) : _CP1 * (m + j + 1)],
                                rhs=pt[:, _CH * j : _CH * (j + 1)],
                                start=(m + j == 0),
                                stop=(m + j == _NMT - 1),
                                skip_group_check=True,
                            )
                        m += msz
                    invden = sp.tile([1, _CH], F32R, name="invden", tag="invden",
                                     bufs=2)
                    # stage PSUM accumulator to SBUF quickly so po frees for
                    # the next chunk, then normalize: y = u[0:64] * (1/u[64])
                    usb = sp.tile([_CP1, _CH], F32, name="usb", tag="usb", bufs=2)
                    nc.vector.tensor_copy(usb[:], po[0:_CP1, :])
                    num, den = usb[0:_C, :], usb[_C : _C + 1, :]
                    with nc.allow_low_precision(
                        reason="1/den rounded to f32r for the broadcast matmul"
                    ):
                        nc.vector.reciprocal(invden[:], den)
                    # broadcast 1/den across partitions with a K=1 matmul
                    bcp = auxp.tile([_C, _CH], F32, name="bcp", tag="aux")
                    nc.tensor.matmul(
                        bcp[:], lhsT=ones_r[:], rhs=invden[:], start=True, stop=True
                    )
                    nc.vector.tensor_tensor(
                        yr[:, _CH * ci : _CH * (ci + 1)], num, bcp[:], op=ALU.mult
                    )
                    # finish the previous chunk (its boundary row needed this y)
                    if ci >= 1:
                        emit_dw(8 * ci - 1, 8 * ci)  # boundary row of chunk ci-1
                        emit_conv3(ci - 1)
                    # depthwise rows that don't need the next chunk's y
                    emit_dw(8 * ci, 8 * ci + 7)
                emit_dw(_N // _W - 1, _N // _W)  # last row (no dy=+1 tap)
                emit_conv3(_NCH - 1)

            if reps == 1:
                emit_all()
            else:
                with tc.For_i(0, reps, 1):
                    emit_all()

    nc.finalize()
    return nc


def _get_nc():
    if "nc" not in _STATE:
        _STATE["nc"] = _build_program()
    return _STATE["nc"]


def _prep_inputs(x, w1, bn1_g, bn1_b, bn1_m, bn1_v,
                 w2, bn2_g, bn2_b, bn2_m, bn2_v,
                 w3, bn3_g, bn3_b, bn3_m, bn3_v):
    f32 = np.float32
    x = np.asarray(x, f32)
    inv1 = np.asarray(bn1_g, f32) / np.sqrt(np.asarray(bn1_v, f32) + _EPS)
    w1p = np.asarray(w1, f32)[:, :, 0, 0] * inv1[:, None]
    b1p = np.asarray(bn1_b, f32) - np.asarray(bn1_m, f32) * inv1
    w1aug = np.concatenate([w1p.T, b1p[None, :]], axis=0)

    inv2 = np.asarray(bn2_g, f32) / np.sqrt(np.asarray(bn2_v, f32) + _EPS)
    w2p = np.asarray(w2, f32)[:, 0].reshape(_C, 9) * inv2[:, None]
    b2p = (np.asarray(bn2_b, f32) - np.asarray(bn2_m, f32) * inv2)[:, None]

    inv3 = np.asarray(bn3_g, f32) / np.sqrt(np.asarray(bn3_v, f32) + _EPS)
    w3p = np.asarray(w3, f32)[:, :, 0, 0] * inv3[:, None]
    b3p = np.asarray(bn3_b, f32) - np.asarray(bn3_m, f32) * inv3
    w3aug = np.concatenate([w3p.T, b3p[None, :]], axis=0)

    consts = np.zeros((_CP1, _NCONST), f32)
    consts[:, 0:64] = w1aug
    consts[:, 64:128] = w3aug
    consts[0:_C, 128:137] = w2p
    consts[0:_C, 137:138] = b2p
    for k in range(9):
        consts[0:_C, 138 + _C * k : 138 + _C * (k + 1)] = np.diag(w2p[:, k])
    consts[0:_C, 714:723] = -w2p

    B = x.shape[0]
    in_maps = []
    for i in range(B):
        in_maps.append({
            "x": np.ascontiguousarray(x[i].reshape(_C, _N)),
            "consts": consts,
        })
    return in_maps


def kernel(**inputs) -> np.ndarray:
    from concourse.bass_utils import run_bass_kernel_spmd

    in_maps = _prep_inputs(**inputs)
    nc = _get_nc()
    _STATE["in_maps"] = in_maps
    res = run_bass_kernel_spmd(nc, in_maps, list(range(len(in_maps))))
    out = np.stack(
        [r["out"].reshape(_C, _W, _W) for r in res.results]
    ).astype(np.float32)
    return out


def profile_exec_time():
    """Re-run the last inputs with NTFF tracing; returns exec time in ns."""
    from concourse.bass_utils import run_bass_kernel_spmd

    nc = _get_nc()
    in_maps = _STATE.get("in_maps")
    assert in_maps is not None, "call kernel() first"
    res = run_bass_kernel_spmd(nc, in_maps, list(range(len(in_maps))), trace=True)
    return res



# revision 8
# speedup vs baseline: 1.6270x; 1.6270x over previous
# Trainium2 Bass kernel for nn_Block_SA (dense_cnn self-attention block).
#
# Per-sample computation (C=64 channels, 64x64 spatial, N=4096 positions):
#   v   = relu(bn1(conv1x1(x)))                      # V for attention
#   s   = (x^T x) / sqrt(C)                          # [N, N] scores, Q=K=x
#   p   = softmax(s, axis=-1)
#   a   = V p^T  (a[d,n] = sum_m p[n,m] V[d,m])
#   z   = relu(bn2(depthwise3x3(a)))
#   out = bn3(conv1x1(z)) + x
#
# Distribution: batch B=8, one sample per NeuronCore (data parallel, no
# collectives). BN params are folded into conv weights on the host.
#
# On-chip algorithm (per core):
#   - Scores are computed TRANSPOSED: sT[m, n] tiles via matmul(lhsT=x[:,mtile],
#     rhs=x[:,nchunk]) so softmax's sum over m becomes a matmul reduction.
#     The K=64 contraction uses only half the PE array, so score matmuls are
#     row-packed two-at-a-time with tile_position (x duplicated on partitions
#     64-127) for ~2x PE throughput.
#   - Score/attention matmul operands are bf16 (PSUM accumulation stays fp32).
#     fp32-mode matmuls trip the HW activity throttle (util limit 0.5 for
#     ~half the run in the fp32r version); bf16 avoids it. Accuracy loss is
#     ~0.5% rel, well under the 2e-2 gate.
#   - exp() without max subtraction (scores/8 are small; fp32 is safe).
#   - The denominator sum_m exp(sT[m,n]) is obtained for free by augmenting
#     V^T with a ones column (row 64 of the AV accumulator).
#   - AV accumulates over all 32 m-tiles into one PSUM bank; normalization:
#     1/den via the fast custom-DVE reciprocal (18 bits), broadcast across
#     partitions on the (idle) Pool engine, multiply on DVE.
#   - Depthwise 3x3 runs on the PE as 9 accumulating diagonal-weight matmuls
#     over shifted views of the attention output (PSUM accumulates taps);
#     the vector engine only applies bias+relu. This and conv3 stay fp32r
#     (long streams run full rate; keeps the non-attention path exact).
#   - Depthwise/conv3 emission is DEFERRED and interleaved into the NEXT
#     chunk's score groups, so the in-order PE queue never stalls waiting
#     for the DVE normalize chain.
#   - conv3 + bias via augmented ones row; residual add in fp32; DMA out.

import numpy as np

_EPS = 1e-5
_C = 64
_CP1 = 65
_N = 4096
_CH = 512          # free-dim chunk (one PSUM bank of fp32)
_NCH = _N // _CH   # 8 chunks
_MT = 128          # m-tile (partition dim of transposed score tiles)
_NMT = _N // _MT   # 32 m-tiles
_W = 64            # image width
_GROUPS = [3] * 10 + [2]   # m-tiles per exp batch (3 PSUM banks per batch)
_NCONST = 138 + 9 * 64 + 9  # w1aug | w3aug | w2p | b2p | diags | -w2p

_STATE = {}


def _build_program(reps=1):
    import concourse.bacc as bacc
    import concourse.tile as tile
    from concourse import mybir

    F32 = mybir.dt.float32
    F32R = mybir.dt.float32r
    BF16 = mybir.dt.bfloat16
    U32 = mybir.dt.uint32
    AF = mybir.ActivationFunctionType
    ALU = mybir.AluOpType
    ONE_BITS = 0x3F800000

    nc = bacc.Bacc(None)

    xd = nc.dram_tensor("x", [_C, _N], F32, kind="ExternalInput")
    # packed weights -> one DMA: cols 0:64 w1aug, 64:128 w3aug,
    # 128:137 w2p (rows 0:64), 137 b2p, 138:714 diag(w2p[:,k]) k=0..8
    cd = nc.dram_tensor("consts", [_CP1, _NCONST], F32, kind="ExternalInput")
    outd = nc.dram_tensor("out", [_C, _N], F32, kind="ExternalOutput")

    with tile.TileContext(nc) as tc:
        with (
            tc.tile_pool(name="persist", bufs=1) as pp,
            tc.tile_pool(name="small", bufs=2) as sp,
            tc.tile_pool(name="pt_pool", bufs=3) as ptp,
            tc.tile_pool(name="ps_pool", bufs=2, space="PSUM") as psp,
            tc.tile_pool(name="po_pool", bufs=1, space="PSUM") as pop,
            tc.tile_pool(name="aux_pool", bufs=1, space="PSUM") as auxp,
        ):
            def emit_all():
                # ---- input staging. x is DMA'd from HBM once (fp32, kept for
                # the residual), cast to bf16 on DVE/Pool, and the bf16 copy is
                # duplicated to partitions 64:128 by SBUF-to-SBUF DMA so score
                # matmuls can row-pack.
                xo = pp.tile([_C, _N], F32, name="xo", tag="xo")
                xa = pp.tile([_CP1, _N], BF16, name="xa", tag="xa")
                xb2 = pp.tile([_MT, _N], BF16, name="xb2", tag="xb2")
                for s in range(4):
                    sl = slice(1024 * s, 1024 * (s + 1))
                    nc.sync.dma_start(xo[:, sl], xd[:, sl])
                    eng = nc.vector if s % 2 == 0 else nc.gpsimd
                    eng.tensor_copy(xa[0:_C, sl], xo[:, sl])
                    nc.sync.dma_start(xb2[_C:_MT, sl], xa[0:_C, sl])
                nc.gpsimd.memset(xa[_C:_CP1, :], 1.0)

                cs = pp.tile([_CP1, _NCONST], F32, name="cs", tag="cs")
                nc.scalar.dma_start(cs[:], cd[:])
                w2s = cs[0:_C, 128:137]
                b2s = cs[0:_C, 137:138]
                w2n = cs[0:_C, 714:723]

                # rounded weight copies (bf16 for conv1, f32r for dw/conv3)
                w1b = pp.tile([_CP1, _C], BF16, name="w1b", tag="w1b")
                nc.vector.tensor_copy(w1b[:], cs[:, 0:64])
                w3r = pp.tile([_CP1, _C], F32R, name="w3r", tag="w3r")
                nc.vector.tensor_copy(w3r[:], cs[:, 64:128])
                dgr = pp.tile([_C, 9 * _C], F32R, name="dgr", tag="dgr")
                nc.gpsimd.tensor_copy(dgr[:], cs[0:_C, 138:138 + 9 * _C])

                # V^T blocks: per m-tile a [128, 65] block (col 64 = ones)
                vt = pp.tile([_MT, _NMT * _CP1], BF16, name="vt", tag="vt")
                vt3 = vt.rearrange("p (t c) -> p t c", c=_CP1)
                nc.gpsimd.memset(vt3[:, :, _C:_CP1], 1.0)

                # normalized attention output (f32r: feeds the PE depthwise).
                # One zeroed pad row of 64 on each side so flat row-spanning
                # shifted reads stay in bounds.
                yrp = pp.tile([_C, _N + 2 * _W], F32R, name="yrp", tag="yrp")
                nc.gpsimd.memset(yrp[:, 0:_W].bitcast(U32), 0)
                nc.gpsimd.memset(yrp[:, _W + _N : _N + 2 * _W].bitcast(U32), 0)
                yr = yrp[:, _W : _W + _N]
                # post-depthwise activations (+ones row) feeding conv3
                zr = pp.tile([_CP1, _N], F32R, name="zr", tag="zr")
                nc.gpsimd.memset(zr[_C:_CP1, :].bitcast(U32), ONE_BITS)
                zrv = zr[0:_C, :].rearrange("c (h w) -> c h w", w=_W)

                # ---- V^T groups: emitted lazily (interleaved into chunk 0's
                # group loop) so the cold PE isn't blocked on them at startup.
                # relu on DVE (not ACT) so the scalar engine runs Exp only.
                _vt_emitted = [0]

                def emit_vt_groups(need_mtiles):
                    while _vt_emitted[0] * 4 < need_mtiles:
                        g = _vt_emitted[0]
                        vps = auxp.tile([_MT, 4 * _C], F32, name="vps", tag="aux")
                        for j in range(4):
                            m = 4 * g + j
                            nc.tensor.matmul(
                                vps[:, _C * j : _C * (j + 1)],
                                lhsT=xa[:, _MT * m : _MT * (m + 1)],
                                rhs=w1b[:],
                                start=True,
                                stop=True,
                            )
                        nc.vector.tensor_relu(
                            vt3[:, 4 * g : 4 * (g + 1), 0:_C],
                            vps[:].rearrange("p (t c) -> p t c", c=_C),
                        )
                        _vt_emitted[0] += 1

                # ---- depthwise 3x3 on the PE: 9 accumulating diagonal matmuls
                # over flat shifted views of yr (full-row spans; the dx=+-1 taps
                # wrap across row edges and get small DVE fix-ups that subtract
                # the wrong contributions), then bias+relu on DVE.
                yrp3 = yrp.rearrange("c (h w) -> c h w", w=_W)  # row i = y row i-1

                def emit_dw(h0, h1):
                    nh = h1 - h0
                    dwp = auxp.tile([_C, nh * _W], F32, name="dwp", tag="aux")
                    dwp3 = dwp.rearrange("c (h w) -> c h w", w=_W)
                    mms = []      # (k, out_lo, out_hi, src_lo, src_hi)
                    fixups = []
                    lastrow = []
                    for k in [4, 0, 1, 2, 3, 5, 6, 7, 8]:
                        dy, dx = k // 3 - 1, k % 3 - 1
                        hh0, hh1 = max(h0, -dy), min(h1, _W - dy)
                        if hh1 <= hh0:
                            continue
                        # tap k=8's flat span would read one element of the NEXT
                        # chunk's y (row hh1+1, w=0): do its last row exactly
                        flat_hh1 = hh1 - 1 if k == 8 else hh1
                        nhh = flat_hh1 - hh0
                        if nhh > 0:
                            src = _W + (hh0 + dy) * _W + dx
                            mms.append((k, (hh0 - h0) * _W, (flat_hh1 - h0) * _W,
                                        src, src + nhh * _W))
                            if dx != 0:
                                fixups.append((k, dy, dx, hh0, nhh))
                        if k == 8:
                            # f32r matmuls need even free counts; this 63-wide
                            # row goes on the DVE instead (after the PE group)
                            lastrow.append(hh1 - 1)
                    for i, (k, o0, o1, s0, s1) in enumerate(mms):
                        nc.tensor.matmul(
                            dwp[:, o0:o1],
                            lhsT=dgr[:, _C * k : _C * (k + 1)],
                            rhs=yrp[:, s0:s1],
                            start=(i == 0),
                            stop=(i == len(mms) - 1),
                            skip_group_check=True,
                        )
                    # tap k=8's last row: out[h, 0:63] += w2[8]*y[h+1, 1:64)
                    for h in lastrow:
                        dst8 = dwp3[:, h - h0 : h - h0 + 1, 0 : _W - 1]
                        nc.vector.scalar_tensor_tensor(
                            dst8,
                            yrp3[:, h + 2 : h + 3, 1:_W],
                            w2s[:, 8:9],
                            dst8,
                            op0=ALU.mult,
                            op1=ALU.add,
                        )
                    # subtract the wrap-around contribution of the dx=+-1 taps:
                    # dx=-1 polluted w=0 (read prev flat row's w=63), dx=+1
                    # polluted w=63 (read next flat row's w=0)
                    for k, dy, dx, hh0, nhh in fixups:
                        if dx == -1:
                            dst = dwp3[:, hh0 - h0 : hh0 - h0 + nhh, 0:1]
                            bad = yrp3[:, hh0 + dy : hh0 + dy + nhh, _W - 1 : _W]
                        else:
                            dst = dwp3[:, hh0 - h0 : hh0 - h0 + nhh, _W - 1 : _W]
                            bad = yrp3[:, hh0 + dy + 2 : hh0 + dy + 2 + nhh, 0:1]
                        nc.vector.scalar_tensor_tensor(
                            dst, bad, w2n[:, k : k + 1], dst, op0=ALU.mult, op1=ALU.add
                        )
                    nc.vector.tensor_scalar(
                        zrv[:, h0:h1, :], dwp3[:], b2s, 0.0, op0=ALU.add, op1=ALU.max
                    )

                def emit_conv3(c):
                    # conv3 (+bias via ones row) + residual + store
                    pc = auxp.tile([_C, _CH], F32, name="pc", tag="aux")
                    nc.tensor.matmul(
                        pc[:],
                        lhsT=w3r[:],
                        rhs=zr[:, _CH * c : _CH * (c + 1)],
                        start=True,
                        stop=True,
                    )
                    outt = sp.tile([_C, _CH], F32, name="outt", tag="outt", bufs=2)
                    nc.vector.tensor_tensor(
                        outt[:], pc[:], xo[:, _CH * c : _CH * (c + 1)], op=ALU.add
                    )
                    nc.sync.dma_start(outd[:, _CH * c : _CH * (c + 1)], outt[:])

                # ---- main fused-attention loop over n-chunks ----
                # Deferred depthwise/conv3 closures, popped between the NEXT
                # chunk's score groups (keeps the in-order PE queue stall-free).
                pending = []
                for ci in range(_NCH):
                    po = pop.tile([_MT, _CH], F32, name="po", tag="po")
                    m = 0
                    # chunk 0 leads with small groups so the scalar engine's exp
                    # stream starts as soon as the first score tile exists
                    groups = ([1, 2] + [3] * 9 + [2]) if ci == 0 else _GROUPS
                    for gi, msz in enumerate(groups):
                        ps = psp.tile([_MT, _CH * msz], F32, name="ps", tag="ps")
                        for j in range(msz):
                            mt = m + j
                            if mt % 2 == 0:
                                src, rows, tp = xa, slice(0, _C), (0, 0)
                            else:
                                src, rows, tp = xb2, slice(_C, _MT), (_C, 0)
                            nc.tensor.matmul(
                                ps[:, _CH * j : _CH * (j + 1)],
                                lhsT=src[rows, _MT * mt : _MT * (mt + 1)],
                                rhs=src[rows, _CH * ci : _CH * (ci + 1)],
                                start=True,
                                stop=True,
                                tile_position=tp,
                            )
                        pt = ptp.tile([_MT, _CH * msz], BF16, name="pt", tag="pt")
                        nc.scalar.activation(pt[:], ps[:], AF.Exp, scale=0.125)
                        if ci == 0:
                            emit_vt_groups(m + msz)
                        for j in range(msz):
                            nc.tensor.matmul(
                                po[0:_CP1, :],
                                lhsT=vt[:, _CP1 * (m + j) : _CP1 * (m + j + 1)],
                                rhs=pt[:, _CH * j : _CH * (j + 1)],
                                start=(m + j == 0),
                                stop=(m + j == _NMT - 1),
                                skip_group_check=True,
                            )
                        m += msz
                        if gi in (4, 8) and pending:
                            pending.pop(0)()
                    # normalize: y = u[0:64] * (1/u[64]). Custom-DVE ops need a
                    # partition-0-aligned source, so first stage the den row to
                    # partition 0 with a plain (shift-capable) DVE copy, then
                    # the fast reciprocal; partition-broadcast on Pool (PE and
                    # ACT stay out of this chain entirely).
                    dsb = sp.tile([1, _CH], F32, name="dsb", tag="dsb", bufs=2)
                    nc.vector.tensor_copy(dsb[:], po[_C : _C + 1, :])
                    invf = sp.tile([1, _CH], F32, name="invf", tag="invf", bufs=2)
                    nc.vector.reciprocal_approx_fast(out=invf[:], in_=dsb[:])
                    bcps = sp.tile([_C, _CH], F32, name="bcps", tag="bcps", bufs=2)
                    nc.gpsimd.partition_broadcast(bcps[:], invf[:])
                    nc.vector.tensor_tensor(
                        yr[:, _CH * ci : _CH * (ci + 1)], po[0:_C, :], bcps[:],
                        op=ALU.mult,
                    )
                    # queue this chunk's depthwise (and finish chunk ci-1:
                    # its boundary row needed this chunk's y)
                    pending.append(lambda ci=ci: emit_dw(8 * ci, 8 * ci + 7))
                    if ci >= 1:
                        def fin(ci=ci):
                            emit_dw(8 * ci - 1, 8 * ci)
                            emit_conv3(ci - 1)
                        pending.append(fin)
                for f in pending:
                    f()
                emit_dw(_N // _W - 1, _N // _W)  # last row (no dy=+1 tap)
                emit_conv3(_NCH - 1)

            if reps == 1:
                emit_all()
            else:
                with tc.For_i(0, reps, 1):
                    emit_all()

    nc.finalize()
    return nc


def _get_nc():
    if "nc" not in _STATE:
        _STATE["nc"] = _build_program()
    return _STATE["nc"]


def _prep_inputs(x, w1, bn1_g, bn1_b, bn1_m, bn1_v,
                 w2, bn2_g, bn2_b, bn2_m, bn2_v,
                 w3, bn3_g, bn3_b, bn3_m, bn3_v):
    f32 = np.float32
    x = np.asarray(x, f32)
    inv1 = np.asarray(bn1_g, f32) / np.sqrt(np.asarray(bn1_v, f32) + _EPS)
    w1p = np.asarray(w1, f32)[:, :, 0, 0] * inv1[:, None]
    b1p = np.asarray(bn1_b, f32) - np.asarray(bn1_m, f32) * inv1
    w1aug = np.concatenate([w1p.T, b1p[None, :]], axis=0)

    inv2 = np.asarray(bn2_g, f32) / np.sqrt(np.asarray(bn2_v, f32) + _EPS)
    w2p = np.asarray(w2, f32)[:, 0].reshape(_C, 9) * inv2[:, None]
    b2p = (np.asarray(bn2_b, f32) - np.asarray(bn2_m, f32) * inv2)[:, None]

    inv3 = np.asarray(bn3_g, f32) / np.sqrt(np.asarray(bn3_v, f32) + _EPS)
    w3p = np.asarray(w3, f32)[:, :, 0, 0] * inv3[:, None]
    b3p = np.asarray(bn3_b, f32) - np.asarray(bn3_m, f32) * inv3
    w3aug = np.concatenate([w3p.T, b3p[None, :]], axis=0)

    consts = np.zeros((_CP1, _NCONST), f32)
    consts[:, 0:64] = w1aug
    consts[:, 64:128] = w3aug
    consts[0:_C, 128:137] = w2p
    consts[0:_C, 137:138] = b2p
    for k in range(9):
        consts[0:_C, 138 + _C * k : 138 + _C * (k + 1)] = np.diag(w2p[:, k])
    consts[0:_C, 714:723] = -w2p

    B = x.shape[0]
    in_maps = []
    for i in range(B):
        in_maps.append({
            "x": np.ascontiguousarray(x[i].reshape(_C, _N)),
            "consts": consts,
        })
    return in_maps


def kernel(**inputs) -> np.ndarray:
    from concourse.bass_utils import run_bass_kernel_spmd

    in_maps = _prep_inputs(**inputs)
    nc = _get_nc()
    _STATE["in_maps"] = in_maps
    res = run_bass_kernel_spmd(nc, in_maps, list(range(len(in_maps))))
    out = np.stack(
        [r["out"].reshape(_C, _W, _W) for r in res.results]
    ).astype(np.float32)
    return out


def profile_exec_time():
    """Re-run the last inputs with NTFF tracing; returns exec time in ns."""
    from concourse.bass_utils import run_bass_kernel_spmd

    nc = _get_nc()
    in_maps = _STATE.get("in_maps")
    assert in_maps is not None, "call kernel() first"
    res = run_bass_kernel_spmd(nc, in_maps, list(range(len(in_maps))), trace=True)
    return res
